# revision 1
# baseline (speedup 1.0000x reference)
"""Boundary rendering module for Trainium2 (8 NeuronCores) — single-launch.

Computes, for x of shape (2, 4, 64, 256, 256) f32:
    mn/mx  = per-channel global min/max
    binary = ((x - mn) / (mx - mn)) > 0.5     [== x > (mn+mx)/2]
    dilated = 3x3x3 binary dilation of binary (SAME padding)
    out    = dilated - binary

Sharding: H (=256) split into 8 chunks of 32 rows, one per NeuronCore, with
one halo row per side (global edges padded with -1e30 -> mask "unset").
On-core layout: partition axis = (B, D) = 128; (C, H, W) on the free axis.

Single NEFF per core, DMA-bandwidth-bound design:
  1. 12 chunked loads of the f32 shard (SWDGE via gpsimd — measured ~1.6x
     faster than HWDGE here — plus sync/scalar rings), staged f32; per chunk
     exact f32 min/max partials on DVE; ACT Copy converts to bf16 image xb.
  2. Per channel: combine partials -> [mx, -mn]; gpsimd partition_all_reduce;
     an 8-byte per-channel AllReduce(max) across the 8 cores (issued as soon
     as that channel's reduce is done, overlapping later loads); gpsimd
     partition_broadcast of the result.
  3. Mask in {-1,+1} via ACT Sign(2*xb + (mn+mx-2t)=...)  [Sign(xb - t)];
     H-dilation on DVE (2 max ops, split per 16-row half for pipelining);
     PE: per 512-col PSUM bank 3 W-shifted matmuls against the banded
     (b,d)-matrix A plus one -16*I matmul on the mask.  With pads = -1 the
     count satisfies: out=1 iff psum >= 18 - 3*rowA (rowA = D-band size,
     2 at d edges else 3), exact in integers.
  4. ACT drains PSUM with a saturated sigmoid (per-partition bias
     600*rowA - 3500, scale 200) producing exact {0,1} in fp8; stores are
     fp8 (4x smaller than f32; host casts back exactly).
"""

import os
import sys

import numpy as np

for _p in ("/opt/trn_rl_repo", "/root/.axon_site/_ro/trn_rl_repo"):
    if os.path.isdir(_p) and _p not in sys.path:
        sys.path.insert(0, _p)

import ml_dtypes

B, C, D, H, W = 2, 4, 64, 256, 256
NCORES = 8
HS = H // NCORES  # 32 own rows per core
HA = HS + 2  # rows incl halo
HPAD = np.float32(-1e30)  # halo pad at global H edges -> mask unset

MHW = 258  # mH row width: 256 data cols + 2 pad cols (-1)
MHLEN = 1 + 32 * MHW + 3  # lead pad + 32 rows + slack for dw=+1 views
HAW = HA * W  # 8704 flat elems per channel per partition
CHUNKS = [(0, 6), (6, 12), (12, 18), (18, 24), (24, 29), (29, 34)]
NCH = len(CHUNKS)  # 6 per channel
STG_ROWS = 6
NSLOT = 6

_CACHE = {}


def _consts():
    bd = np.arange(128)
    b = bd // D
    d = bd % D
    A = (b[:, None] == b[None, :]) & (np.abs(d[:, None] - d[None, :]) <= 1)
    rowA = A.sum(1).astype(np.float32)  # 2 at d edges, else 3
    A = A.astype(ml_dtypes.bfloat16)
    negI = (-16.0 * np.eye(128)).astype(ml_dtypes.bfloat16)
    dbias = (600.0 * rowA - 3500.0).astype(np.float32).reshape(128, 1)
    return A, negI, dbias


def _build(reps: int = 1, parts: str = "all", dbg: bool = False):
    import concourse.bass as bass
    import concourse.bacc as bacc
    import concourse.mybir as mybir
    import concourse.tile as tile
    import concourse.bass_isa as bass_isa
    from contextlib import ExitStack

    f32 = mybir.dt.float32
    bf16 = mybir.dt.bfloat16
    fp8 = mybir.dt.float8e4
    Alu = mybir.AluOpType
    Act = mybir.ActivationFunctionType

    on = lambda p: parts == "all" or p in parts

    nc = bacc.Bacc(
        "TRN2",
        target_bir_lowering=False,
        debug=False,
        num_devices=NCORES,
    )

    xs = nc.dram_tensor("xs", [B, C, D, HA, W], f32, kind="ExternalInput")
    out = nc.dram_tensor("out", [B, C, D, HS, W], fp8, kind="ExternalOutput")
    dbg_t = nc.dram_tensor("dbg", [128, 2064], f32, kind="ExternalOutput") if dbg else None
    A_np, negI_np, dbias_np = _consts()
    bandA_d = nc.inline_tensor(A_np, name="bandA")
    negI_d = nc.inline_tensor(negI_np, name="negI")
    dbias_d = nc.inline_tensor(dbias_np, name="dbias")

    xsa = xs.ap()
    outa = out.ap()

    with ExitStack() as ctx:
        tc = ctx.enter_context(tile.TileContext(nc))
        pers = ctx.enter_context(tc.tile_pool(name="pers", bufs=1))
        psump = ctx.enter_context(tc.tile_pool(name="psum", bufs=2, space="PSUM"))
        dram = ctx.enter_context(tc.tile_pool(name="dram", bufs=1, space="DRAM"))

        xb = pers.tile([128, C * HAW], bf16)  # 68 KiB bf16 image of x
        stgl = [
            pers.tile([128, STG_ROWS * W], f32, name=f"stg{i}")
            for i in range(NSLOT)
        ]
        binm0 = pers.tile([128, HAW], bf16)  # 17 KiB {-1,+1} mask
        binm1 = pers.tile([128, HAW], bf16)
        m1 = pers.tile([128, 34 * W], bf16)  # H-dil intermediate (t-halves)
        mH0 = pers.tile([128, MHLEN], bf16)  # H-dilated, padded with -1
        mH1 = pers.tile([128, MHLEN], bf16)
        stag0 = pers.tile([128, 4096], fp8)  # 4 KiB out staging (16 rows)
        stag1 = pers.tile([128, 4096], fp8)
        pmax = pers.tile([128, 24], f32)
        pmin = pers.tile([128, 24], f32)
        red8 = pers.tile([128, 8], f32)  # per-channel [mx, -mn] pairs
        par8 = pers.tile([128, 8], f32)  # partition-allreduced pairs
        s1v = pers.tile([128, 8], f32)  # AllReduce results (partition 0)
        gv8 = pers.tile([128, 8], f32)  # broadcast pairs on all partitions
        negt = pers.tile([128, 4], f32)  # -2*t_c per channel (Sign bias)
        At = pers.tile([128, 128], bf16)
        Nt = pers.tile([128, 128], bf16)
        dbias = pers.tile([128, 1], f32)
        dbgb = pers.tile([128, 2064], f32, name="dbgb") if dbg else None

        ar_ins = [
            dram.tile([1, 2], f32, name=f"ar_in{r}")
            for r in range(reps * C)
        ]
        ar_outs = [
            dram.tile([1, 2], f32, addr_space="Shared", name=f"ar_out{r}")
            for r in range(reps * C)
        ]

        stgs = tuple(stgl)
        binms = (binm0, binm1)
        mHs = (mH0, mH1)
        stags = (stag0, stag1)

        nc.vector.memset(mH0[:, :], -1.0)  # pads -1 = "unset"; data rewritten
        nc.vector.memset(mH1[:, :], -1.0)
        nc.sync.dma_start(out=At[:, :], in_=bandA_d.ap())
        nc.sync.dma_start(out=Nt[:, :], in_=negI_d.ap())
        nc.sync.dma_start(out=dbias[:, :], in_=dbias_d.ap())
        if parts != "all":
            for t in (xb, *stgl, binm0, binm1, m1, stag0, stag1,
                      pmax, pmin, red8, par8, s1v, gv8, negt):
                nc.vector.memset(t[:, :], 0.0)

        for _rep in range(reps):
            def _chunk(c, j):
                k = NCH * c + j
                r0, r1 = CHUNKS[j]
                stg = stgs[k % NSLOT]
                if on("dma"):
                    nc.gpsimd.dma_start(
                        out=stg[:, 0 : (r1 - r0) * W],
                        in_=xsa[:, c, :, r0:r1, :],
                    )
                if on("red"):
                    lo = max(1, r0) - r0
                    hi = min(33, r1) - r0
                    rview = stg[:, lo * W : hi * W]
                    nc.vector.tensor_reduce(
                        out=pmax[:, k : k + 1],
                        in_=rview,
                        axis=mybir.AxisListType.X,
                        op=Alu.max,
                    )
                    nc.vector.tensor_reduce(
                        out=pmin[:, k : k + 1],
                        in_=rview,
                        axis=mybir.AxisListType.X,
                        op=Alu.min,
                    )
                if on("conv"):
                    nc.scalar.activation(
                        out=xb[:, c * HAW + r0 * W : c * HAW + r1 * W],
                        in_=stg[:, 0 : (r1 - r0) * W],
                        func=Act.Copy,
                    )

            def _combine(c):
                if not on("red"):
                    return
                nc.vector.tensor_reduce(
                    out=red8[:, 2 * c : 2 * c + 1],
                    in_=pmax[:, NCH * c : NCH * c + NCH],
                    axis=mybir.AxisListType.X,
                    op=Alu.max,
                )
                nc.vector.tensor_reduce(
                    out=red8[:, 2 * c + 1 : 2 * c + 2],
                    in_=pmin[:, NCH * c : NCH * c + NCH],
                    axis=mybir.AxisListType.X,
                    op=Alu.min,
                )
                nc.vector.tensor_scalar_mul(
                    red8[:, 2 * c + 1 : 2 * c + 2],
                    red8[:, 2 * c + 1 : 2 * c + 2],
                    -1.0,
                )

            def _ar(c):
                if not on("ar"):
                    return
                sl = slice(2 * c, 2 * c + 2)
                nc.gpsimd.partition_all_reduce(
                    par8[:, sl], red8[:, sl], 128, bass_isa.ReduceOp.max
                )
                nc.gpsimd.dma_start(
                    out=ar_ins[_rep * C + c][:, :], in_=par8[0:1, sl]
                )
                nc.gpsimd.collective_compute(
                    "AllReduce",
                    Alu.max,
                    replica_groups=[list(range(NCORES))],
                    ins=[ar_ins[_rep * C + c].opt()],
                    outs=[ar_outs[_rep * C + c].opt()],
                )

            # loads c0; loads c1; ar c0; loads c2; ar c1; loads c3; ar c2; ar c3
            for j in range(NCH):
                _chunk(0, j)
            _combine(0)
            for j in range(NCH):
                _chunk(1, j)
            _combine(1)
            _ar(0)
            for j in range(NCH):
                _chunk(2, j)
            _combine(2)
            _ar(1)
            for j in range(NCH):
                _chunk(3, j)
            _combine(3)
            _ar(2)
            _ar(3)

            if on("ar"):
                for c in range(C):
                    sl = slice(2 * c, 2 * c + 2)
                    nc.gpsimd.dma_start(
                        out=s1v[0:1, sl], in_=ar_outs[_rep * C + c][:, :]
                    )
                    nc.gpsimd.partition_broadcast(gv8[:, sl], s1v[0:1, sl])
                for c in range(C):
                    # Sign bias: -2*t = (-mn) - mx  (activation scale = 2)
                    nc.vector.tensor_tensor(
                        out=negt[:, c : c + 1],
                        in0=gv8[:, 2 * c + 1 : 2 * c + 2],
                        in1=gv8[:, 2 * c : 2 * c + 1],
                        op=Alu.subtract,
                    )

            # ---- per channel: mask, then per t: dilate, count, drain, store
            for c in range(C):
                bi = c % 2
                binm = binms[bi]
                mH = mHs[bi]
                if on("mask"):
                    nc.scalar.activation(
                        out=binm[:, :],
                        in_=xb[:, c * HAW : (c + 1) * HAW],
                        func=Act.Sign,
                        bias=negt[:, c : c + 1],
                        scale=2.0,
                    )
                for t in range(2):
                    if on("dil"):
                        # rows 16t..16t+15 of mH need binm rows 16t..16t+17
                        rb = 16 * t
                        # m1 rows rb..rb+16 (17 rows)
                        nc.vector.tensor_tensor(
                            out=m1[:, rb * W : (rb + 17) * W],
                            in0=binm[:, rb * W : (rb + 17) * W],
                            in1=binm[:, (rb + 1) * W : (rb + 18) * W],
                            op=Alu.max,
                        )
                        mHd = mH[:, 1 + rb * MHW : 1 + (rb + 16) * MHW].rearrange(
                            "p (r z) -> p r z", z=MHW
                        )[:, :, 0:W]
                        nc.vector.tensor_tensor(
                            out=mHd,
                            in0=m1[:, rb * W : (rb + 16) * W].rearrange(
                                "p (r z) -> p r z", z=W
                            ),
                            in1=m1[:, (rb + 1) * W : (rb + 17) * W].rearrange(
                                "p (r z) -> p r z", z=W
                            ),
                            op=Alu.max,
                        )
                    stag = stags[(2 * c + t) % 2]
                    ps = psump.tile([128, 2048], f32, tag="ps")
                    ps2 = psump.tile([128, 2048], f32, tag="ps")
                    for half, pst_ in ((0, ps), (1, ps2)):
                        if on("pe"):
                            for s in range(4):
                                R = 16 * t + 8 * half + 2 * s
                                pslice = pst_[:, 512 * s : 512 * s + 512]
                                for j, dw in enumerate((-1, 0, 1)):
                                    off = 1 + R * MHW + dw
                                    rhs = mH[:, off : off + 2 * MHW].rearrange(
                                        "p (r z) -> p r z", z=MHW
                                    )[:, :, 0:W]
                                    nc.tensor.matmul(
                                        pslice,
                                        At[:, :],
                                        rhs,
                                        start=(j == 0),
                                        stop=False,
                                    )
                                nc.tensor.matmul(
                                    pslice,
                                    Nt[:, :],
                                    binm[:, (R + 1) * W : (R + 3) * W],
                                    start=False,
                                    stop=True,
                                )
                        if on("drain"):
                            nc.scalar.activation(
                                out=stag[:, 2048 * half : 2048 * half + 2048],
                                in_=pst_[:, :],
                                func=Act.Sigmoid,
                                bias=dbias[:, :],
                                scale=200.0,
                            )
                    if on("store"):
                        nc.sync.dma_start(
                            out=outa[:, c, :, 16 * t : 16 * t + 16, :],
                            in_=stag.rearrange("p (r w) -> p r w", w=W),
                        )
                    if dbg and c == 0 and t == 0:
                        # gv8(8), negt(4), binm rows0-1(512), mH row0(258),
                        # psum bank0 (512), stag half0 (512), dbias(1)
                        nc.vector.tensor_copy(out=dbgb[:, 0:8], in_=gv8[:, :])
                        nc.vector.tensor_copy(out=dbgb[:, 8:12], in_=negt[:, :])
                        nc.vector.tensor_copy(out=dbgb[:, 16:528], in_=binm[:, 0:512])
                        nc.vector.tensor_copy(out=dbgb[:, 528:786], in_=mH[:, 0:258])
                        nc.vector.tensor_copy(out=dbgb[:, 786:1298], in_=ps[:, 0:512])
                        nc.vector.tensor_copy(out=dbgb[:, 1298:1810], in_=stag[:, 0:512])
                        nc.vector.tensor_copy(out=dbgb[:, 1810:1811], in_=dbias[:, :])
                        nc.sync.dma_start(out=dbg_t.ap(), in_=dbgb[:, :])

    nc.compile()
    return nc


def _get_nc(reps: int = 1, parts: str = "all", dbg: bool = False):
    key = (reps, parts, dbg)
    if key not in _CACHE:
        _CACHE[key] = _build(reps=reps, parts=parts, dbg=dbg)
    return _CACHE[key]


def _make_in_maps(x: np.ndarray):
    in_maps = []
    for k in range(NCORES):
        xsh = np.empty((B, C, D, HA, W), np.float32)
        lo = k * HS
        xsh[:, :, :, 1 : HS + 1, :] = x[:, :, :, lo : lo + HS, :]
        if k > 0:
            xsh[:, :, :, 0, :] = x[:, :, :, lo - 1, :]
        else:
            xsh[:, :, :, 0, :] = HPAD
        if k < NCORES - 1:
            xsh[:, :, :, HS + 1, :] = x[:, :, :, lo + HS, :]
        else:
            xsh[:, :, :, HS + 1, :] = HPAD
        in_maps.append({"xs": xsh})
    return in_maps


def kernel(x: np.ndarray) -> np.ndarray:
    from concourse.bass_utils import run_bass_kernel_spmd

    x = np.ascontiguousarray(np.asarray(x), dtype=np.float32)
    assert x.shape == (B, C, D, H, W)

    in_maps = _make_in_maps(x)
    res = run_bass_kernel_spmd(_get_nc(), in_maps, core_ids=list(range(NCORES)))
    pieces = [
        np.asarray(res.results[k]["out"]).astype(np.float32)
        for k in range(NCORES)
    ]
    return np.concatenate(pieces, axis=3)


if __name__ == "__main__":
    x = np.random.randn(B, C, D, H, W).astype(np.float32)
    y = kernel(x)
    print(y.shape, y.dtype, y.sum())

